# revision 12
# baseline (speedup 1.0000x reference)
# Trainium2 Bass kernel for nn_Attention_80779744903426
#
# Reference computation (b=4, n=2048, c=1024, h=16, d=64):
#   qkv = x @ w_qkv ; split to q,k,v per head
#   attn = softmax(q k^T / sqrt(c)) ; out = (attn v) concat ; y = out @ w_proj + b_proj
#
# Sharding (8 cores): data-parallel over batch (4) x tensor-parallel over
# head-groups (2 groups of 8 heads, Megatron-style). Each core computes a
# partial y for its batch from its 8 heads; host sums the two partials per
# batch and adds b_proj.
#
# Per-core program:
#   A) (fp32r) qk^T = wqk^T @ x^T -> staged to DRAM as bf16 [1024, 2048]
#      (Q^T rows 0:512, K^T rows 512:1024). Two passes: K^T/Q^T for pair 0
#      first so phase B can start early; V = x @ wv -> SBUF bf16 in pass 2,
#      stored per (k-tile, head) with a ones column appended.
#   B) (bf16) per head pair and q-chunk of 512:
#      S^T[k,q] = K^T_h(stationary, row-tiled K=64) x Q^T_h(moving); head A/B
#      matmuls interleaved so they run concurrently on different row groups;
#      exp via ACT over 3-bank PSUM batches (scale folded), written bf16;
#      O'[65,q] = [V_h | 1]^T @ P~^T accumulated over 16 k-tiles (fused
#      softmax denominator); normalize rows 0:64 by row 64 via DVE
#      (reciprocal + partition-broadcast DMA via DRAM bounce + multiply)
#   C) (bf16) y = O^T(stationary) @ wp(moving), accumulated over 4 o-tiles.

import numpy as np

DIM = 1024
N = 2048
B = 4
NH = 16
HD = 64
SCALE = 1.0 / DIM**0.5

HPC = 8            # heads per core
PAIRS = HPC // 2   # head pairs (row-tiled together)
CT = 8             # contraction tiles over c=1024
NT = 16            # n tiles of 128
ACH = 512          # phase-A n-chunk
QCH = 512          # phase-B q-chunk
NQC = N // QCH     # 4 q-chunks
KT = 16            # k tiles of 128 in attention

S_BATCHES = [(0, 3), (3, 3), (6, 3), (9, 3), (12, 2), (14, 2)]

_CACHE = {}


def _build_nc():
    import concourse.bass as bass
    from concourse import bacc, mybir, tile

    f32 = mybir.dt.float32
    f32r = mybir.dt.float32r
    bf16 = mybir.dt.bfloat16
    EXP = mybir.ActivationFunctionType.Exp

    def mmr(out, lhsT, rhs, **kw):  # fp32r matmul on fp32 storage
        nc.tensor.matmul(out, lhsT.bitcast(f32r), rhs.bitcast(f32r), **kw)

    nc = bacc.Bacc("TRN2", target_bir_lowering=False, debug=False)

    xT_d = nc.dram_tensor("xT", [DIM, N], f32, kind="ExternalInput").ap()
    wqk_d = nc.dram_tensor("wqk", [DIM, 1024], f32, kind="ExternalInput").ap()
    wv_d = nc.dram_tensor("wv", [DIM, 512], f32, kind="ExternalInput").ap()
    wp_d = nc.dram_tensor("wp", [512, DIM], bf16, kind="ExternalInput").ap()
    y_d = nc.dram_tensor("y", [N, DIM], f32, kind="ExternalOutput").ap()

    with tile.TileContext(nc) as tc:
        with (
            tc.tile_pool(name="p16", bufs=4) as p16,      # 16KB slots: xT chunks <-> P~ tiles
            tc.tile_pool(name="wqk", bufs=1) as wqkp,     # 32KB
            tc.tile_pool(name="wv", bufs=1) as wvp,       # 16KB
            tc.tile_pool(name="wp", bufs=1) as wpp,       # 8KB bf16
            tc.tile_pool(name="v", bufs=1) as vp,         # 16.6KB bf16
            tc.tile_pool(name="ot", bufs=1) as otp,       # 16KB bf16
            tc.tile_pool(name="kt", bufs=2) as ktp,       # 4KB x2 bf16
            tc.tile_pool(name="qt", bufs=2) as qtp,       # 1KB x2 bf16
            tc.tile_pool(name="misc", bufs=2) as miscp,
            tc.tile_pool(name="ps", bufs=1, space="PSUM") as psp,
            tc.tile_pool(name="dram", bufs=1, space="DRAM") as dp,
        ):
            qkT_d = dp.tile([DIM, N], bf16, name="qkT_stage")
            # ---- static tiles ----
            wqk_sb = wqkp.tile([128, CT, 1024], f32)
            for ct in range(CT):
                nc.sync.dma_start(wqk_sb[:, ct, :].bitcast(f32r),
                                  wqk_d[128 * ct : 128 * (ct + 1), :].bitcast(f32r))
            wv_sb = wvp.tile([128, CT, 512], f32)
            for ct in range(CT):
                nc.sync.dma_start(wv_sb[:, ct, :].bitcast(f32r),
                                  wv_d[128 * ct : 128 * (ct + 1), :].bitcast(f32r))
            wp_sb = wpp.tile([128, 4, 1024], bf16)
            for ot in range(4):
                nc.sync.dma_start(wp_sb[:, ot, :], wp_d[128 * ot : 128 * (ot + 1), :])

            v_sb = vp.tile([128, NT, HPC, HD + 1], bf16)  # [k-part, k-tile, head, d | 1]
            nc.vector.memset(v_sb[:, :, :, HD], 1.0)

            ot_sb = otp.tile([128, PAIRS, N], bf16)  # O^T rows: pair p = rows 128p..

            xT_r = xT_d.rearrange("(t p) n -> p t n", p=128)

            # ---- phase A: qkv projections (fp32r) ----
            # mt 0..3 = Q^T pairs, mt 4..7 = K^T pairs. Pass 1 stages pair-0/1
            # Q^T,K^T so phase B starts early; pass 2 does the rest + V.
            for mts, do_v in [([4, 0, 5, 1], False), ([6, 2, 7, 3], True)]:
                for ach in range(N // ACH):
                    xt = p16.tile([128, CT, ACH], f32, tag="big16", name="xt")
                    nc.sync.dma_start(xt.bitcast(f32r),
                                      xT_r[:, :, ACH * ach : ACH * (ach + 1)].bitcast(f32r))
                    for mt in mts:
                        qps = psp.tile([128, 512], f32, tag="acc", bufs=2, name="qps")
                        for ct in range(CT):
                            mmr(qps, wqk_sb[:, ct, 128 * mt : 128 * (mt + 1)],
                                xt[:, ct, :], start=(ct == 0), stop=(ct == CT - 1))
                        stg = miscp.tile([128, 512], bf16, tag="stg", bufs=3, name="stg")
                        nc.vector.tensor_copy(stg, qps)
                        nc.sync.dma_start(
                            qkT_d[128 * mt : 128 * (mt + 1), ACH * ach : ACH * (ach + 1)],
                            stg,
                        )
                    if do_v:
                        # V = x @ wv : out [n-tile, 512] -> v_sb (bf16 cast)
                        for sub in range(ACH // 128):
                            nt = (ACH // 128) * ach + sub
                            vps = psp.tile([128, 512], f32, tag="acc", bufs=2, name="vps")
                            for ct in range(CT):
                                mmr(vps, xt[:, ct, 128 * sub : 128 * (sub + 1)],
                                    wv_sb[:, ct, :], start=(ct == 0), stop=(ct == CT - 1))
                            nc.vector.tensor_copy(
                                v_sb[:, nt, :, 0:HD],
                                vps.rearrange("p (h d) -> p h d", h=HPC),
                            )

            # ---- phase B: attention per head-pair (bf16) ----
            for p in range(PAIRS):
                kt_sb = ktp.tile([128, N], bf16)  # K^T rows for both heads of the pair
                nc.sync.dma_start(kt_sb, qkT_d[512 + 128 * p : 512 + 128 * (p + 1), :])
                for qc in range(NQC):
                    qt_sb = qtp.tile([128, QCH], bf16)
                    nc.sync.dma_start(
                        qt_sb, qkT_d[128 * p : 128 * (p + 1), QCH * qc : QCH * (qc + 1)]
                    )
                    ptiles = [
                        p16.tile([128, KT, QCH], bf16, tag="big16", name=f"pt{hh}")
                        for hh in range(2)
                    ]
                    # S^T + exp: head A (rows 0:64) and head B (rows 64:128)
                    # interleaved so consecutive matmuls hit different row
                    # groups and run concurrently.
                    for b0, bn in S_BATCHES:
                        sps = [
                            psp.tile([128, 3, QCH], f32, tag="sb3", bufs=2,
                                     name=f"sps{hh}")
                            for hh in range(2)
                        ]
                        for i in range(bn):
                            k = b0 + i
                            for hh in range(2):
                                sl = slice(64 * hh, 64 * (hh + 1))
                                nc.tensor.matmul(
                                    sps[hh][:, i, :],
                                    kt_sb[sl, 128 * k : 128 * (k + 1)],
                                    qt_sb[sl, :], start=True, stop=True)
                        for hh in range(2):
                            nc.scalar.activation(
                                out=ptiles[hh][:, b0 : b0 + bn, :],
                                in_=sps[hh][:, 0:bn, :],
                                func=EXP,
                                scale=float(SCALE),
                            )
                    # O' = [V | 1]^T @ P~^T, then normalize by fused row sums
                    for hh in range(2):
                        h = 2 * p + hh
                        ops = psp.tile([HD + 1, QCH], f32, tag="acc", bufs=2, name="ops")
                        for k in range(KT):
                            nc.tensor.matmul(ops, v_sb[:, k, h, :], ptiles[hh][:, k, :],
                                             start=(k == 0), stop=(k == KT - 1))
                        rcp = miscp.tile([1, QCH], f32, tag="rcp")
                        nc.vector.reciprocal(rcp, ops[HD : HD + 1, :])
                        # partition-broadcast via DRAM bounce (stride-0 partition
                        # APs are only legal on DRAM sources)
                        rcp_d = dp.tile([1, QCH], f32, tag="rcpd", bufs=4, name="rcpd")
                        nc.sync.dma_start(rcp_d, rcp)
                        bc = miscp.tile([64, QCH], f32, tag="bc")
                        rap = rcp_d[:]
                        nc.sync.dma_start(
                            bc,
                            bass.AP(tensor=rap.tensor, offset=rap.offset,
                                    ap=[[0, 64]] + list(rap.ap[1:])),
                        )
                        nc.vector.tensor_mul(
                            ot_sb[64 * hh : 64 * (hh + 1), p, QCH * qc : QCH * (qc + 1)],
                            ops[0:HD, :],
                            bc,
                        )

            # ---- phase C: y = O @ wp (bf16) ----
            for nt in range(NT):
                for yc in range(2):
                    yps = psp.tile([128, 512], f32, tag="acc", bufs=2, name="yps")
                    for ot in range(4):
                        nc.tensor.matmul(
                            yps, ot_sb[:, ot, 128 * nt : 128 * (nt + 1)],
                            wp_sb[:, ot, 512 * yc : 512 * (yc + 1)],
                            start=(ot == 0), stop=(ot == 3))
                    stg = miscp.tile([128, 512], f32, tag="ystg", bufs=2, name="ystg")
                    nc.vector.tensor_copy(stg, yps)
                    nc.sync.dma_start(
                        y_d[128 * nt : 128 * (nt + 1), 512 * yc : 512 * (yc + 1)], stg
                    )

    nc.compile()
    return nc


def get_nc():
    if "nc" not in _CACHE:
        _CACHE["nc"] = _build_nc()
    return _CACHE["nc"]


def make_in_maps(x, w_qkv, w_proj):
    import ml_dtypes

    in_maps = []
    for c in range(8):
        b, g = c // 2, c % 2
        in_maps.append({
            "xT": np.ascontiguousarray(x[b].T, dtype=np.float32),
            "wqk": np.ascontiguousarray(
                np.concatenate(
                    [w_qkv[:, 512 * g : 512 * (g + 1)],
                     w_qkv[:, 1024 + 512 * g : 1024 + 512 * (g + 1)]], axis=1
                ), dtype=np.float32),
            "wv": np.ascontiguousarray(
                w_qkv[:, 2048 + 512 * g : 2048 + 512 * (g + 1)], dtype=np.float32),
            "wp": np.ascontiguousarray(
                w_proj[512 * g : 512 * (g + 1), :]).astype(ml_dtypes.bfloat16),
        })
    return in_maps


def kernel(x, w_qkv, w_proj, b_proj):
    from concourse.bass_utils import run_bass_kernel_spmd

    x = np.asarray(x, dtype=np.float32)
    w_qkv = np.asarray(w_qkv, dtype=np.float32)
    w_proj = np.asarray(w_proj, dtype=np.float32)
    b_proj = np.asarray(b_proj, dtype=np.float32)

    nc = get_nc()
    in_maps = make_in_maps(x, w_qkv, w_proj)
    res = run_bass_kernel_spmd(nc, in_maps, list(range(8))).results

    out = np.zeros((B, N, DIM), dtype=np.float32)
    for c in range(8):
        out[c // 2] += res[c]["y"]
    return out + b_proj


# revision 13
# speedup vs baseline: 1.0714x; 1.0714x over previous
# Trainium2 Bass kernel for nn_Attention_80779744903426
#
# Reference computation (b=4, n=2048, c=1024, h=16, d=64):
#   qkv = x @ w_qkv ; split to q,k,v per head
#   attn = softmax(q k^T / sqrt(c)) ; out = (attn v) concat ; y = out @ w_proj + b_proj
#
# Sharding (8 cores): data-parallel over batch (4) x tensor-parallel over
# head-groups (2 groups of 8 heads, Megatron-style). Each core computes a
# partial y for its batch from its 8 heads; host sums the two partials per
# batch and adds b_proj.
#
# Per-core program:
#   A) (fp32r) qk^T = wqk^T @ x^T -> staged to DRAM as bf16 [1024, 2048]
#      (Q^T rows 0:512, K^T rows 512:1024). Two passes: K^T/Q^T for pair 0
#      first so phase B can start early; V = x @ wv -> SBUF bf16 in pass 2,
#      stored per (k-tile, head) with a ones column appended.
#   B) (bf16) per head pair and q-chunk of 512:
#      S^T[k,q] = K^T_h(stationary, row-tiled K=64) x Q^T_h(moving); head A/B
#      matmuls interleaved so they run concurrently on different row groups;
#      exp via ACT over 3-bank PSUM batches (scale folded), written bf16;
#      O'[65,q] = [V_h | 1]^T @ P~^T accumulated over 16 k-tiles (fused
#      softmax denominator); normalize rows 0:64 by row 64 via DVE
#      (reciprocal + partition-broadcast DMA via DRAM bounce + multiply)
#   C) (bf16) y = O^T(stationary) @ wp(moving), accumulated over 4 o-tiles.

import numpy as np

DIM = 1024
N = 2048
B = 4
NH = 16
HD = 64
SCALE = 1.0 / DIM**0.5

HPC = 8            # heads per core
PAIRS = HPC // 2   # head pairs (row-tiled together)
CT = 8             # contraction tiles over c=1024
NT = 16            # n tiles of 128
ACH = 512          # phase-A n-chunk
QCH = 512          # phase-B q-chunk
NQC = N // QCH     # 4 q-chunks
KT = 16            # k tiles of 128 in attention

S_BATCHES = [(0, 3), (3, 3), (6, 3), (9, 3), (12, 2), (14, 2)]

_CACHE = {}


def _build_nc():
    import concourse.bass as bass
    from concourse import bacc, mybir, tile

    f32 = mybir.dt.float32
    f32r = mybir.dt.float32r
    bf16 = mybir.dt.bfloat16
    EXP = mybir.ActivationFunctionType.Exp

    def mmr(out, lhsT, rhs, **kw):  # fp32r matmul on fp32 storage
        nc.tensor.matmul(out, lhsT.bitcast(f32r), rhs.bitcast(f32r), **kw)

    nc = bacc.Bacc("TRN2", target_bir_lowering=False, debug=False)

    xT_d = nc.dram_tensor("xT", [DIM, N], f32, kind="ExternalInput").ap()
    wqk_d = nc.dram_tensor("wqk", [DIM, 1024], f32, kind="ExternalInput").ap()
    wv_d = nc.dram_tensor("wv", [DIM, 512], f32, kind="ExternalInput").ap()
    wp_d = nc.dram_tensor("wp", [512, DIM], bf16, kind="ExternalInput").ap()
    y_d = nc.dram_tensor("y", [N, DIM], f32, kind="ExternalOutput").ap()

    with tile.TileContext(nc) as tc:
        with (
            tc.tile_pool(name="p16", bufs=4) as p16,      # 16KB slots: xT chunks <-> P~ tiles
            tc.tile_pool(name="wqk", bufs=1) as wqkp,     # 32KB
            tc.tile_pool(name="wv", bufs=1) as wvp,       # 16KB
            tc.tile_pool(name="wp", bufs=1) as wpp,       # 8KB bf16
            tc.tile_pool(name="v", bufs=1) as vp,         # 16.6KB bf16
            tc.tile_pool(name="ot", bufs=1) as otp,       # 16KB bf16
            tc.tile_pool(name="kt", bufs=2) as ktp,       # 4KB x2 bf16
            tc.tile_pool(name="qt", bufs=2) as qtp,       # 1KB x2 bf16
            tc.tile_pool(name="misc", bufs=2) as miscp,
            tc.tile_pool(name="ps", bufs=1, space="PSUM") as psp,
            tc.tile_pool(name="dram", bufs=1, space="DRAM") as dp,
        ):
            qkT_d = dp.tile([DIM, N], bf16, name="qkT_stage")
            # ---- static tiles ----
            wqk_sb = wqkp.tile([128, CT, 1024], f32)
            for ct in range(CT):
                nc.sync.dma_start(wqk_sb[:, ct, :].bitcast(f32r),
                                  wqk_d[128 * ct : 128 * (ct + 1), :].bitcast(f32r))
            wv_sb = wvp.tile([128, CT, 512], f32)
            for ct in range(CT):
                nc.sync.dma_start(wv_sb[:, ct, :].bitcast(f32r),
                                  wv_d[128 * ct : 128 * (ct + 1), :].bitcast(f32r))
            wp_sb = wpp.tile([128, 4, 1024], bf16)
            for ot in range(4):
                nc.sync.dma_start(wp_sb[:, ot, :], wp_d[128 * ot : 128 * (ot + 1), :])

            v_sb = vp.tile([128, NT, HPC, HD + 1], bf16)  # [k-part, k-tile, head, d | 1]
            nc.vector.memset(v_sb[:, :, :, HD], 1.0)

            ot_sb = otp.tile([128, PAIRS, N], bf16)  # O^T rows: pair p = rows 128p..

            xT_r = xT_d.rearrange("(t p) n -> p t n", p=128)

            # ---- phase A: qkv projections (fp32r) ----
            # mt 0..3 = Q^T pairs, mt 4..7 = K^T pairs. Pass 1 stages pair-0/1
            # Q^T,K^T so phase B starts early; pass 2 does the rest + V.
            for mts, do_v in [([4, 0, 5, 1], False), ([6, 2, 7, 3], True)]:
                for ach in range(N // ACH):
                    xt = p16.tile([128, CT, ACH], f32, tag="big16", name="xt")
                    nc.sync.dma_start(xt.bitcast(f32r),
                                      xT_r[:, :, ACH * ach : ACH * (ach + 1)].bitcast(f32r))
                    for mt in mts:
                        qps = psp.tile([128, 512], f32, tag="acc", bufs=2, name="qps")
                        for ct in range(CT):
                            mmr(qps, wqk_sb[:, ct, 128 * mt : 128 * (mt + 1)],
                                xt[:, ct, :], start=(ct == 0), stop=(ct == CT - 1))
                        stg = miscp.tile([128, 512], bf16, tag="stg", bufs=3, name="stg")
                        nc.vector.tensor_copy(stg, qps)
                        nc.sync.dma_start(
                            qkT_d[128 * mt : 128 * (mt + 1), ACH * ach : ACH * (ach + 1)],
                            stg,
                        )
                    if do_v:
                        # V = x @ wv : out [n-tile, 512] -> v_sb (bf16 cast)
                        for sub in range(ACH // 128):
                            nt = (ACH // 128) * ach + sub
                            vps = psp.tile([128, 512], f32, tag="acc", bufs=2, name="vps")
                            for ct in range(CT):
                                mmr(vps, xt[:, ct, 128 * sub : 128 * (sub + 1)],
                                    wv_sb[:, ct, :], start=(ct == 0), stop=(ct == CT - 1))
                            nc.vector.tensor_copy(
                                v_sb[:, nt, :, 0:HD],
                                vps.rearrange("p (h d) -> p h d", h=HPC),
                            )

            # ---- phase B: attention per head-pair (bf16), software-pipelined ----
            # The PE executes its queue in order, so an S matmul stalling on an
            # exp-slot WAR would idle the PE (and HAM re-throttles the clock).
            # Interleave PV segments of the PREVIOUS (pair, q-chunk) between the
            # S batches of the current one so the PE stream never blocks.
            PV_SEGS = [(0, 3), (3, 3), (6, 3), (9, 3), (12, 2), (14, 2)]

            def emit_pv_segment(st, seg):
                p0, ptl, opsl = st
                k0, kn = PV_SEGS[seg]
                for hh in range(2):
                    h = 2 * p0 + hh
                    for k in range(k0, k0 + kn):
                        nc.tensor.matmul(opsl[hh], v_sb[:, k, h, :],
                                         ptl[hh][:, k, :],
                                         start=(k == 0), stop=(k == KT - 1))

            def emit_norm(st, qc0):
                p0, ptl, opsl = st
                for hh in range(2):
                    ops = opsl[hh]
                    rcp = miscp.tile([1, QCH], f32, tag="rcp", name="rcp")
                    nc.vector.reciprocal(rcp, ops[HD : HD + 1, :])
                    # partition-broadcast via DRAM bounce (stride-0 partition
                    # APs are only legal on DRAM sources)
                    rcp_d = dp.tile([1, QCH], f32, tag="rcpd", bufs=4, name="rcpd")
                    nc.sync.dma_start(rcp_d, rcp)
                    bc = miscp.tile([64, QCH], f32, tag="bc", name="bc")
                    rap = rcp_d[:]
                    nc.sync.dma_start(
                        bc,
                        bass.AP(tensor=rap.tensor, offset=rap.offset,
                                ap=[[0, 64]] + list(rap.ap[1:])),
                    )
                    nc.vector.tensor_mul(
                        ot_sb[64 * hh : 64 * (hh + 1), p0, QCH * qc0 : QCH * (qc0 + 1)],
                        ops[0:HD, :],
                        bc,
                    )

            pv_st = None   # (pair, ptiles, ops tiles) awaiting PV+norm
            pv_qc = None
            for p in range(PAIRS):
                kt_sb = ktp.tile([128, N], bf16, name="kt_sb")
                nc.sync.dma_start(kt_sb, qkT_d[512 + 128 * p : 512 + 128 * (p + 1), :])
                for qc in range(NQC):
                    qt_sb = qtp.tile([128, QCH], bf16, name="qt_sb")
                    nc.sync.dma_start(
                        qt_sb, qkT_d[128 * p : 128 * (p + 1), QCH * qc : QCH * (qc + 1)]
                    )
                    ptiles = [
                        p16.tile([128, KT, QCH], bf16, tag="big16", name=f"pt{hh}")
                        for hh in range(2)
                    ]
                    for bi, (b0, bn) in enumerate(S_BATCHES):
                        sps = [
                            psp.tile([128, 3, QCH], f32, tag="sb3", bufs=2,
                                     name=f"sps{hh}")
                            for hh in range(2)
                        ]
                        # head A (rows 0:64) / head B (rows 64:128) interleaved:
                        # consecutive matmuls hit different row groups and run
                        # concurrently.
                        for i in range(bn):
                            k = b0 + i
                            for hh in range(2):
                                sl = slice(64 * hh, 64 * (hh + 1))
                                nc.tensor.matmul(
                                    sps[hh][:, i, :],
                                    kt_sb[sl, 128 * k : 128 * (k + 1)],
                                    qt_sb[sl, :], start=True, stop=True)
                        for hh in range(2):
                            nc.scalar.activation(
                                out=ptiles[hh][:, b0 : b0 + bn, :],
                                in_=sps[hh][:, 0:bn, :],
                                func=EXP,
                                scale=float(SCALE),
                            )
                        if pv_st is not None:
                            emit_pv_segment(pv_st, bi)
                    if pv_st is not None:
                        emit_norm(pv_st, pv_qc)
                    opsl = [
                        psp.tile([HD + 1, QCH], f32, tag="acc", bufs=2, name=f"ops{hh}")
                        for hh in range(2)
                    ]
                    pv_st = (p, ptiles, opsl)
                    pv_qc = qc
            # drain the last (pair, q-chunk)
            for seg in range(len(PV_SEGS)):
                emit_pv_segment(pv_st, seg)
            emit_norm(pv_st, pv_qc)

            # ---- phase C: y = O @ wp (bf16) ----
            for nt in range(NT):
                for yc in range(2):
                    yps = psp.tile([128, 512], f32, tag="acc", bufs=2, name="yps")
                    for ot in range(4):
                        nc.tensor.matmul(
                            yps, ot_sb[:, ot, 128 * nt : 128 * (nt + 1)],
                            wp_sb[:, ot, 512 * yc : 512 * (yc + 1)],
                            start=(ot == 0), stop=(ot == 3))
                    stg = miscp.tile([128, 512], f32, tag="ystg", bufs=2, name="ystg")
                    nc.vector.tensor_copy(stg, yps)
                    nc.sync.dma_start(
                        y_d[128 * nt : 128 * (nt + 1), 512 * yc : 512 * (yc + 1)], stg
                    )

    nc.compile()
    return nc


def get_nc():
    if "nc" not in _CACHE:
        _CACHE["nc"] = _build_nc()
    return _CACHE["nc"]


def make_in_maps(x, w_qkv, w_proj):
    import ml_dtypes

    in_maps = []
    for c in range(8):
        b, g = c // 2, c % 2
        in_maps.append({
            "xT": np.ascontiguousarray(x[b].T, dtype=np.float32),
            "wqk": np.ascontiguousarray(
                np.concatenate(
                    [w_qkv[:, 512 * g : 512 * (g + 1)],
                     w_qkv[:, 1024 + 512 * g : 1024 + 512 * (g + 1)]], axis=1
                ), dtype=np.float32),
            "wv": np.ascontiguousarray(
                w_qkv[:, 2048 + 512 * g : 2048 + 512 * (g + 1)], dtype=np.float32),
            "wp": np.ascontiguousarray(
                w_proj[512 * g : 512 * (g + 1), :]).astype(ml_dtypes.bfloat16),
        })
    return in_maps


def kernel(x, w_qkv, w_proj, b_proj):
    from concourse.bass_utils import run_bass_kernel_spmd

    x = np.asarray(x, dtype=np.float32)
    w_qkv = np.asarray(w_qkv, dtype=np.float32)
    w_proj = np.asarray(w_proj, dtype=np.float32)
    b_proj = np.asarray(b_proj, dtype=np.float32)

    nc = get_nc()
    in_maps = make_in_maps(x, w_qkv, w_proj)
    res = run_bass_kernel_spmd(nc, in_maps, list(range(8))).results

    out = np.zeros((B, N, DIM), dtype=np.float32)
    for c in range(8):
        out[c // 2] += res[c]["y"]
    return out + b_proj


# revision 15
# speedup vs baseline: 1.4675x; 1.3697x over previous
# Trainium2 Bass kernel for nn_Attention_80779744903426
#
# Reference computation (b=4, n=2048, c=1024, h=16, d=64):
#   qkv = x @ w_qkv ; split to q,k,v per head
#   attn = softmax(q k^T / sqrt(c)) ; out = (attn v) concat ; y = out @ w_proj + b_proj
#
# Sharding (8 cores): data-parallel over batch (4) x tensor-parallel over
# head-groups (2 groups of 8 heads, Megatron-style). Each core computes a
# partial y for its batch from its 8 heads; host sums the two partials per
# batch and adds b_proj.
#
# Per-core program:
#   A) (fp32r) qk^T = wqk^T @ x^T -> staged to DRAM as bf16 [1024, 2048]
#      (Q^T rows 0:512, K^T rows 512:1024). Two passes: K^T/Q^T for pair 0
#      first so phase B can start early; V = x @ wv -> SBUF bf16 in pass 2,
#      stored per (k-tile, head) with a ones column appended.
#   B) (bf16) per head pair and q-chunk of 512:
#      S^T[k,q] = K^T_h(stationary, row-tiled K=64) x Q^T_h(moving); head A/B
#      matmuls interleaved so they run concurrently on different row groups;
#      exp via ACT over 3-bank PSUM batches (scale folded), written bf16;
#      O'[65,q] = [V_h | 1]^T @ P~^T accumulated over 16 k-tiles (fused
#      softmax denominator); normalize rows 0:64 by row 64 via DVE
#      (reciprocal + partition-broadcast DMA via DRAM bounce + multiply)
#   C) (bf16) y = O^T(stationary) @ wp(moving), accumulated over 4 o-tiles.

import numpy as np

DIM = 1024
N = 2048
B = 4
NH = 16
HD = 64
SCALE = 1.0 / DIM**0.5

HPC = 8            # heads per core
PAIRS = HPC // 2   # head pairs (row-tiled together)
CT = 8             # contraction tiles over c=1024
NT = 16            # n tiles of 128
ACH = 512          # phase-A n-chunk
QCH = 512          # phase-B q-chunk
NQC = N // QCH     # 4 q-chunks
KT = 16            # k tiles of 128 in attention

S_BATCHES = [(0, 3), (3, 3), (6, 3), (9, 3), (12, 2), (14, 2)]

_CACHE = {}


def _build_nc():
    import concourse.bass as bass
    from concourse import bacc, mybir, tile

    f32 = mybir.dt.float32
    f32r = mybir.dt.float32r
    bf16 = mybir.dt.bfloat16
    EXP = mybir.ActivationFunctionType.Exp

    def mmr(out, lhsT, rhs, **kw):  # fp32r matmul on fp32 storage
        nc.tensor.matmul(out, lhsT.bitcast(f32r), rhs.bitcast(f32r), **kw)

    nc = bacc.Bacc("TRN2", target_bir_lowering=False, debug=False)

    xT_d = nc.dram_tensor("xT", [DIM, N], f32, kind="ExternalInput").ap()
    wqk_d = nc.dram_tensor("wqk", [DIM, 1024], f32, kind="ExternalInput").ap()
    wv_d = nc.dram_tensor("wv", [DIM, 512], f32, kind="ExternalInput").ap()
    wp_d = nc.dram_tensor("wp", [512, DIM], bf16, kind="ExternalInput").ap()
    y_d = nc.dram_tensor("y", [N, DIM], f32, kind="ExternalOutput").ap()

    with tile.TileContext(nc) as tc:
        with (
            tc.tile_pool(name="p16", bufs=4) as p16,      # 16KB slots: xT chunks <-> P~ tiles
            tc.tile_pool(name="wqk", bufs=1) as wqkp,     # 32KB
            tc.tile_pool(name="wv", bufs=1) as wvp,       # 16KB
            tc.tile_pool(name="wp", bufs=1) as wpp,       # 8KB bf16
            tc.tile_pool(name="v", bufs=1) as vp,         # 16.6KB bf16
            tc.tile_pool(name="ot", bufs=1) as otp,       # 16KB bf16
            tc.tile_pool(name="kt", bufs=2) as ktp,       # 4KB x2 bf16
            tc.tile_pool(name="qt", bufs=2) as qtp,       # 1KB x2 bf16
            tc.tile_pool(name="misc", bufs=2) as miscp,
            tc.tile_pool(name="ps", bufs=1, space="PSUM") as psp,
            tc.tile_pool(name="dram", bufs=1, space="DRAM") as dp,
        ):
            qkT_d = dp.tile([DIM, N], bf16, name="qkT_stage")
            # ---- static tiles ----
            wqk_sb = wqkp.tile([128, CT, 1024], f32)
            for ct in range(CT):
                nc.sync.dma_start(wqk_sb[:, ct, :].bitcast(f32r),
                                  wqk_d[128 * ct : 128 * (ct + 1), :].bitcast(f32r))
            wv_sb = wvp.tile([128, CT, 512], f32)
            for ct in range(CT):
                nc.sync.dma_start(wv_sb[:, ct, :].bitcast(f32r),
                                  wv_d[128 * ct : 128 * (ct + 1), :].bitcast(f32r))
            wp_sb = wpp.tile([128, 4, 1024], bf16)
            for ot in range(4):
                nc.sync.dma_start(wp_sb[:, ot, :], wp_d[128 * ot : 128 * (ot + 1), :])

            v_sb = vp.tile([128, NT, HPC, HD + 1], bf16)  # [k-part, k-tile, head, d | 1]
            nc.vector.memset(v_sb[:, :, :, HD], 1.0)

            ot_sb = otp.tile([128, PAIRS, N], bf16)  # O^T rows: pair p = rows 128p..

            xT_r = xT_d.rearrange("(t p) n -> p t n", p=128)

            # ---- phase A: qkv projections (fp32r) ----
            # mt 0..3 = Q^T pairs, mt 4..7 = K^T pairs. Pass 1 stages pair-0/1
            # Q^T,K^T so phase B starts early; pass 2 does the rest + V.
            for mts, do_v in [([4, 0, 5, 1], False), ([6, 2, 7, 3], True)]:
                for ach in range(N // ACH):
                    xt = p16.tile([128, CT, ACH], f32, tag="big16", name="xt")
                    nc.sync.dma_start(xt.bitcast(f32r),
                                      xT_r[:, :, ACH * ach : ACH * (ach + 1)].bitcast(f32r))
                    for mt in mts:
                        qps = psp.tile([128, 512], f32, tag="acc", bufs=2, name="qps")
                        for ct in range(CT):
                            mmr(qps, wqk_sb[:, ct, 128 * mt : 128 * (mt + 1)],
                                xt[:, ct, :], start=(ct == 0), stop=(ct == CT - 1))
                        stg = miscp.tile([128, 512], bf16, tag="stg", bufs=3, name="stg")
                        nc.vector.tensor_copy(stg, qps)
                        nc.sync.dma_start(
                            qkT_d[128 * mt : 128 * (mt + 1), ACH * ach : ACH * (ach + 1)],
                            stg,
                        )
                    if do_v:
                        # V = x @ wv : out [n-tile, 512] -> v_sb (bf16 cast)
                        for sub in range(ACH // 128):
                            nt = (ACH // 128) * ach + sub
                            vps = psp.tile([128, 512], f32, tag="acc", bufs=2, name="vps")
                            for ct in range(CT):
                                mmr(vps, xt[:, ct, 128 * sub : 128 * (sub + 1)],
                                    wv_sb[:, ct, :], start=(ct == 0), stop=(ct == CT - 1))
                            nc.vector.tensor_copy(
                                v_sb[:, nt, :, 0:HD],
                                vps.rearrange("p (h d) -> p h d", h=HPC),
                            )

            # ---- phase B: attention per head-pair (bf16), software-pipelined ----
            # The PE executes its queue in order, so an S matmul stalling on an
            # exp-slot WAR would idle the PE (and HAM re-throttles the clock).
            # Interleave PV segments of the PREVIOUS (pair, q-chunk) between the
            # S batches of the current one so the PE stream never blocks.
            PV_SEGS = [(0, 3), (3, 3), (6, 3), (9, 3), (12, 2), (14, 2)]

            def emit_pv_segment(st, seg):
                p0, ptl, opsl = st
                k0, kn = PV_SEGS[seg]
                for hh in range(2):
                    h = 2 * p0 + hh
                    for k in range(k0, k0 + kn):
                        nc.tensor.matmul(opsl[hh], v_sb[:, k, h, :],
                                         ptl[hh][:, k, :],
                                         start=(k == 0), stop=(k == KT - 1))

            def emit_norm(st, qc0):
                # Copy O' out of PSUM first so the PSUM slot recycles without
                # waiting for the (long-latency) reciprocal/broadcast chain.
                # Bounce DMAs ride the gpsimd SWDGE queue so they never
                # head-of-line-block the sync queue carrying bulk loads.
                p0, ptl, opsl = st
                for hh in range(2):
                    ops = opsl[hh]
                    ostg = miscp.tile([HD, QCH], f32, tag="ostg", bufs=4,
                                      name="ostg")
                    nc.vector.tensor_copy(ostg, ops[0:HD, :])
                    # denominator row staged to partition 0: the custom-DVE
                    # reciprocal_approx_fast misreads non-zero base partitions
                    den = miscp.tile([1, QCH], f32, tag="den", bufs=4, name="den")
                    nc.vector.tensor_copy(den, ops[HD : HD + 1, :])
                    rcp = miscp.tile([1, QCH], f32, tag="rcp", bufs=4, name="rcp")
                    nc.vector.reciprocal_approx_fast(rcp, den)
                    # partition-broadcast via DRAM bounce (stride-0 partition
                    # APs are only legal on DRAM sources)
                    rcp_d = dp.tile([1, QCH], f32, tag="rcpd", bufs=4, name="rcpd")
                    nc.gpsimd.dma_start(rcp_d, rcp)
                    bc = miscp.tile([64, QCH], f32, tag="bc", bufs=4, name="bc")
                    rap = rcp_d[:]
                    nc.gpsimd.dma_start(
                        bc,
                        bass.AP(tensor=rap.tensor, offset=rap.offset,
                                ap=[[0, 64]] + list(rap.ap[1:])),
                    )
                    nc.vector.tensor_mul(
                        ot_sb[64 * hh : 64 * (hh + 1), p0, QCH * qc0 : QCH * (qc0 + 1)],
                        ostg,
                        bc,
                    )

            pv_st = None   # (pair, ptiles, ops tiles) awaiting PV+norm
            pv_qc = None
            for p in range(PAIRS):
                kt_sb = ktp.tile([128, N], bf16, name="kt_sb")
                nc.sync.dma_start(kt_sb, qkT_d[512 + 128 * p : 512 + 128 * (p + 1), :])
                for qc in range(NQC):
                    qt_sb = qtp.tile([128, QCH], bf16, name="qt_sb")
                    nc.sync.dma_start(
                        qt_sb, qkT_d[128 * p : 128 * (p + 1), QCH * qc : QCH * (qc + 1)]
                    )
                    ptiles = [
                        p16.tile([128, KT, QCH], bf16, tag="big16", name=f"pt{hh}")
                        for hh in range(2)
                    ]
                    for bi, (b0, bn) in enumerate(S_BATCHES):
                        sps = [
                            psp.tile([128, 3, QCH], f32, tag="sb3", bufs=2,
                                     name=f"sps{hh}")
                            for hh in range(2)
                        ]
                        # head A (rows 0:64) / head B (rows 64:128) interleaved:
                        # consecutive matmuls hit different row groups and run
                        # concurrently.
                        for i in range(bn):
                            k = b0 + i
                            for hh in range(2):
                                sl = slice(64 * hh, 64 * (hh + 1))
                                nc.tensor.matmul(
                                    sps[hh][:, i, :],
                                    kt_sb[sl, 128 * k : 128 * (k + 1)],
                                    qt_sb[sl, :], start=True, stop=True)
                        for hh in range(2):
                            nc.scalar.activation(
                                out=ptiles[hh][:, b0 : b0 + bn, :],
                                in_=sps[hh][:, 0:bn, :],
                                func=EXP,
                                scale=float(SCALE),
                            )
                        if pv_st is not None:
                            emit_pv_segment(pv_st, bi)
                    if pv_st is not None:
                        emit_norm(pv_st, pv_qc)
                    opsl = [
                        psp.tile([HD + 1, QCH], f32, tag="acc", bufs=2, name=f"ops{hh}")
                        for hh in range(2)
                    ]
                    pv_st = (p, ptiles, opsl)
                    pv_qc = qc
            # drain the last (pair, q-chunk)
            for seg in range(len(PV_SEGS)):
                emit_pv_segment(pv_st, seg)
            emit_norm(pv_st, pv_qc)

            # ---- phase C: y = O @ wp (bf16) ----
            for nt in range(NT):
                for yc in range(2):
                    yps = psp.tile([128, 512], f32, tag="acc", bufs=2, name="yps")
                    for ot in range(4):
                        nc.tensor.matmul(
                            yps, ot_sb[:, ot, 128 * nt : 128 * (nt + 1)],
                            wp_sb[:, ot, 512 * yc : 512 * (yc + 1)],
                            start=(ot == 0), stop=(ot == 3))
                    stg = miscp.tile([128, 512], f32, tag="ystg", bufs=2, name="ystg")
                    nc.vector.tensor_copy(stg, yps)
                    nc.sync.dma_start(
                        y_d[128 * nt : 128 * (nt + 1), 512 * yc : 512 * (yc + 1)], stg
                    )

    nc.compile()
    return nc


def get_nc():
    if "nc" not in _CACHE:
        _CACHE["nc"] = _build_nc()
    return _CACHE["nc"]


def make_in_maps(x, w_qkv, w_proj):
    import ml_dtypes

    in_maps = []
    for c in range(8):
        b, g = c // 2, c % 2
        in_maps.append({
            "xT": np.ascontiguousarray(x[b].T, dtype=np.float32),
            "wqk": np.ascontiguousarray(
                np.concatenate(
                    [w_qkv[:, 512 * g : 512 * (g + 1)],
                     w_qkv[:, 1024 + 512 * g : 1024 + 512 * (g + 1)]], axis=1
                ), dtype=np.float32),
            "wv": np.ascontiguousarray(
                w_qkv[:, 2048 + 512 * g : 2048 + 512 * (g + 1)], dtype=np.float32),
            "wp": np.ascontiguousarray(
                w_proj[512 * g : 512 * (g + 1), :]).astype(ml_dtypes.bfloat16),
        })
    return in_maps


def kernel(x, w_qkv, w_proj, b_proj):
    from concourse.bass_utils import run_bass_kernel_spmd

    x = np.asarray(x, dtype=np.float32)
    w_qkv = np.asarray(w_qkv, dtype=np.float32)
    w_proj = np.asarray(w_proj, dtype=np.float32)
    b_proj = np.asarray(b_proj, dtype=np.float32)

    nc = get_nc()
    in_maps = make_in_maps(x, w_qkv, w_proj)
    res = run_bass_kernel_spmd(nc, in_maps, list(range(8))).results

    out = np.zeros((B, N, DIM), dtype=np.float32)
    for c in range(8):
        out[c // 2] += res[c]["y"]
    return out + b_proj


# revision 16
# speedup vs baseline: 1.4976x; 1.0205x over previous
# Trainium2 Bass kernel for nn_Attention_80779744903426
#
# Reference computation (b=4, n=2048, c=1024, h=16, d=64):
#   qkv = x @ w_qkv ; split to q,k,v per head
#   attn = softmax(q k^T / sqrt(c)) ; out = (attn v) concat ; y = out @ w_proj + b_proj
#
# Sharding (8 cores): data-parallel over batch (4) x tensor-parallel over
# head-groups (2 groups of 8 heads, Megatron-style). Each core computes a
# partial y for its batch from its 8 heads; host sums the two partials per
# batch and adds b_proj.
#
# Per-core program (all matmuls bf16, fp32 PSUM accumulation):
#   A) qk^T = wqk^T @ x^T staged to DRAM bf16 (Q^T rows 0:512, K^T rows
#      512:1024); V = x @ wv -> SBUF bf16 with a ones column appended.
#      Pass 1 = V + K^T/Q^T of head-pair 0; the remaining projection chains
#      are interleaved into pair-0's attention iterations so the PE fills
#      the ACT(exp)-bound stretches.
#   B) per head pair and q-chunk of 512, software-pipelined:
#      S^T[k,q] = K^T_h(stationary, row-tiled K=64, A/B heads interleaved on
#      row groups) x Q^T_h(moving); exp via ACT over 3-bank PSUM batches
#      (softmax scale folded into the activation), bf16 out;
#      O'[65,q] = [V_h | 1]^T @ P~^T over 16 k-tiles (ones column = fused
#      softmax denominator). PV of the previous (pair,chunk) is interleaved
#      between S batches of the current one so the in-order PE queue never
#      stalls (stalls re-throttle the PE clock via HAM). Normalization:
#      copy O' out of PSUM, fast-reciprocal of the sums row (partition 0),
#      partition-broadcast via a DRAM-bounce DMA on the gpsimd queue,
#      multiply.
#   C) y = O^T(stationary) @ wp(moving) over 4 o-tiles; interleaved into the
#      last pair's iterations per q-chunk.

import numpy as np

DIM = 1024
N = 2048
B = 4
NH = 16
HD = 64
SCALE = 1.0 / DIM**0.5

HPC = 8            # heads per core
PAIRS = HPC // 2   # head pairs (row-tiled together)
CT = 8             # contraction tiles over c=1024
NT = 16            # n tiles of 128
ACH = 512          # phase-A n-chunk
QCH = 512          # phase-B q-chunk
NQC = N // QCH     # 4 q-chunks
KT = 16            # k tiles of 128 in attention

S_BATCHES = [(0, 3), (3, 3), (6, 3), (9, 3), (12, 2), (14, 2)]

_CACHE = {}


def _build_nc():
    import concourse.bass as bass
    from concourse import bacc, mybir, tile

    f32 = mybir.dt.float32
    bf16 = mybir.dt.bfloat16
    EXP = mybir.ActivationFunctionType.Exp

    nc = bacc.Bacc("TRN2", target_bir_lowering=False, debug=False)

    xT_d = nc.dram_tensor("xT", [DIM, N], bf16, kind="ExternalInput").ap()
    wqk_d = nc.dram_tensor("wqk", [DIM, 1024], bf16, kind="ExternalInput").ap()
    wv_d = nc.dram_tensor("wv", [DIM, 512], bf16, kind="ExternalInput").ap()
    wp_d = nc.dram_tensor("wp", [512, DIM], bf16, kind="ExternalInput").ap()
    y_d = nc.dram_tensor("y", [N, DIM], f32, kind="ExternalOutput").ap()

    with tile.TileContext(nc) as tc:
        with (
            tc.tile_pool(name="p16", bufs=5) as p16,      # 16KB slots: ptiles / xt
            tc.tile_pool(name="wqk", bufs=1) as wqkp,
            tc.tile_pool(name="wv", bufs=1) as wvp,
            tc.tile_pool(name="wp", bufs=1) as wpp,
            tc.tile_pool(name="v", bufs=1) as vp,
            tc.tile_pool(name="ot", bufs=1) as otp,
            tc.tile_pool(name="kt", bufs=2) as ktp,
            tc.tile_pool(name="qt", bufs=2) as qtp,
            tc.tile_pool(name="misc", bufs=2) as miscp,
            tc.tile_pool(name="ps", bufs=1, space="PSUM") as psp,
            tc.tile_pool(name="dram", bufs=1, space="DRAM") as dp,
        ):
            qkT_d = dp.tile([DIM, N], bf16, name="qkT_stage")
            # ---- static tiles ----
            wqk_sb = wqkp.tile([128, CT, 1024], bf16)
            for ct in range(CT):
                nc.sync.dma_start(wqk_sb[:, ct, :], wqk_d[128 * ct : 128 * (ct + 1), :])
            wv_sb = wvp.tile([128, CT, 512], bf16)
            for ct in range(CT):
                nc.sync.dma_start(wv_sb[:, ct, :], wv_d[128 * ct : 128 * (ct + 1), :])
            wp_sb = wpp.tile([128, 4, 1024], bf16)
            for ot in range(4):
                nc.sync.dma_start(wp_sb[:, ot, :], wp_d[128 * ot : 128 * (ot + 1), :])

            v_sb = vp.tile([128, NT, HPC, HD + 1], bf16)  # [k-part, k-tile, head, d | 1]
            nc.vector.memset(v_sb[:, :, :, HD], 1.0)

            ot_sb = otp.tile([128, PAIRS, N], bf16)  # O^T rows: pair p = rows 128p..

            xT_r = xT_d.rearrange("(t p) n -> p t n", p=128)

            # ---- phase A helpers ----
            def emit_qkt_chain(xt, mt, ach):
                qps = psp.tile([128, 512], f32, tag="acc", bufs=2, name="qps")
                for ct in range(CT):
                    nc.tensor.matmul(qps, wqk_sb[:, ct, 128 * mt : 128 * (mt + 1)],
                                     xt[:, ct, :], start=(ct == 0), stop=(ct == CT - 1))
                stg = miscp.tile([128, 512], bf16, tag="stg", bufs=3, name="stg")
                nc.vector.tensor_copy(stg, qps)
                nc.sync.dma_start(
                    qkT_d[128 * mt : 128 * (mt + 1), ACH * ach : ACH * (ach + 1)], stg
                )

            def load_xt(ach):
                xt = p16.tile([128, CT, ACH], bf16, tag="big16", name="xt")
                nc.sync.dma_start(xt, xT_r[:, :, ACH * ach : ACH * (ach + 1)])
                return xt

            # ---- phase A pass 1: V + K^T/Q^T for pair 0 ----
            for ach in range(N // ACH):
                xt = load_xt(ach)
                for sub in range(ACH // 128):
                    nt = (ACH // 128) * ach + sub
                    vps = psp.tile([128, 512], f32, tag="acc", bufs=2, name="vps")
                    for ct in range(CT):
                        nc.tensor.matmul(vps, xt[:, ct, 128 * sub : 128 * (sub + 1)],
                                         wv_sb[:, ct, :], start=(ct == 0),
                                         stop=(ct == CT - 1))
                    nc.vector.tensor_copy(
                        v_sb[:, nt, :, 0:HD],
                        vps.rearrange("p (h d) -> p h d", h=HPC),
                    )
                emit_qkt_chain(xt, 4, ach)
                emit_qkt_chain(xt, 0, ach)

            # ---- phase B (+ interleaved A pass 2 and phase C) ----
            PV_SEGS = [(0, 3), (3, 3), (6, 3), (9, 3), (12, 2), (14, 2)]
            PASS2_MTS = [5, 1, 6, 2, 7, 3]

            def emit_pv_segment(st, seg):
                p0, ptl, opsl = st
                k0, kn = PV_SEGS[seg]
                for hh in range(2):
                    h = 2 * p0 + hh
                    for k in range(k0, k0 + kn):
                        nc.tensor.matmul(opsl[hh], v_sb[:, k, h, :],
                                         ptl[hh][:, k, :],
                                         start=(k == 0), stop=(k == KT - 1))

            def emit_norm(st, qc0):
                # Copy O' out of PSUM first so the PSUM slot recycles without
                # waiting for the reciprocal/broadcast chain. Bounce DMAs ride
                # the gpsimd SWDGE queue so they never head-of-line-block the
                # sync queue carrying bulk loads.
                p0, ptl, opsl = st
                for hh in range(2):
                    ops = opsl[hh]
                    ostg = miscp.tile([HD, QCH], f32, tag="ostg", bufs=4,
                                      name="ostg")
                    nc.vector.tensor_copy(ostg, ops[0:HD, :])
                    # denominator row staged to partition 0: the custom-DVE
                    # reciprocal_approx_fast misreads non-zero base partitions
                    den = miscp.tile([1, QCH], f32, tag="den", bufs=4, name="den")
                    nc.vector.tensor_copy(den, ops[HD : HD + 1, :])
                    rcp = miscp.tile([1, QCH], f32, tag="rcp", bufs=4, name="rcp")
                    nc.vector.reciprocal_approx_fast(rcp, den)
                    rcp_d = dp.tile([1, QCH], f32, tag="rcpd", bufs=4, name="rcpd")
                    nc.gpsimd.dma_start(rcp_d, rcp)
                    bc = miscp.tile([64, QCH], f32, tag="bc", bufs=4, name="bc")
                    rap = rcp_d[:]
                    nc.gpsimd.dma_start(
                        bc,
                        bass.AP(tensor=rap.tensor, offset=rap.offset,
                                ap=[[0, 64]] + list(rap.ap[1:])),
                    )
                    nc.vector.tensor_mul(
                        ot_sb[64 * hh : 64 * (hh + 1), p0, QCH * qc0 : QCH * (qc0 + 1)],
                        ostg,
                        bc,
                    )

            def emit_proj_chunk(qc0):
                # y columns for q-chunk qc0 (needs ot_sb[:, :, chunk] complete)
                for sub in range(QCH // 128):
                    nt = (QCH // 128) * qc0 + sub
                    for yc in range(2):
                        yps = psp.tile([128, 512], f32, tag="acc", bufs=2, name="yps")
                        for ot in range(4):
                            nc.tensor.matmul(
                                yps, ot_sb[:, ot, 128 * nt : 128 * (nt + 1)],
                                wp_sb[:, ot, 512 * yc : 512 * (yc + 1)],
                                start=(ot == 0), stop=(ot == 3))
                        stg = miscp.tile([128, 512], f32, tag="ystg", bufs=2,
                                         name="ystg")
                        nc.vector.tensor_copy(stg, yps)
                        nc.sync.dma_start(
                            y_d[128 * nt : 128 * (nt + 1), 512 * yc : 512 * (yc + 1)],
                            stg,
                        )

            pv_st = None
            pv_qc = None
            for p in range(PAIRS):
                kt_sb = ktp.tile([128, N], bf16, name="kt_sb")
                nc.sync.dma_start(kt_sb, qkT_d[512 + 128 * p : 512 + 128 * (p + 1), :])
                for qc in range(NQC):
                    qt_sb = qtp.tile([128, QCH], bf16, name="qt_sb")
                    nc.sync.dma_start(
                        qt_sb, qkT_d[128 * p : 128 * (p + 1), QCH * qc : QCH * (qc + 1)]
                    )
                    ptiles = [
                        p16.tile([128, KT, QCH], bf16, tag="big16", name=f"pt{hh}")
                        for hh in range(2)
                    ]
                    for bi, (b0, bn) in enumerate(S_BATCHES):
                        sps = [
                            psp.tile([128, 3, QCH], f32, tag="sb3", bufs=2,
                                     name=f"sps{hh}")
                            for hh in range(2)
                        ]
                        for i in range(bn):
                            k = b0 + i
                            for hh in range(2):
                                sl = slice(64 * hh, 64 * (hh + 1))
                                nc.tensor.matmul(
                                    sps[hh][:, i, :],
                                    kt_sb[sl, 128 * k : 128 * (k + 1)],
                                    qt_sb[sl, :], start=True, stop=True)
                        for hh in range(2):
                            nc.scalar.activation(
                                out=ptiles[hh][:, b0 : b0 + bn, :],
                                in_=sps[hh][:, 0:bn, :],
                                func=EXP,
                                scale=float(SCALE),
                            )
                        if pv_st is not None:
                            emit_pv_segment(pv_st, bi)
                    if pv_st is not None:
                        emit_norm(pv_st, pv_qc)
                        if pv_st[0] == PAIRS - 1:
                            emit_proj_chunk(pv_qc)
                    opsl = [
                        psp.tile([HD + 1, QCH], f32, tag="acc", bufs=2,
                                 name=f"ops{hh}")
                        for hh in range(2)
                    ]
                    pv_st = (p, ptiles, opsl)
                    pv_qc = qc
                    if p == 0:
                        # interleave remaining qk^T staging into pair-0's
                        # ACT-bound iterations
                        xt = load_xt(qc)
                        for mt in PASS2_MTS:
                            emit_qkt_chain(xt, mt, qc)
            # drain the last (pair, q-chunk)
            for seg in range(len(PV_SEGS)):
                emit_pv_segment(pv_st, seg)
            emit_norm(pv_st, pv_qc)
            emit_proj_chunk(pv_qc)

    nc.compile()
    return nc


def get_nc():
    if "nc" not in _CACHE:
        _CACHE["nc"] = _build_nc()
    return _CACHE["nc"]


def make_in_maps(x, w_qkv, w_proj):
    import ml_dtypes

    bf = ml_dtypes.bfloat16
    in_maps = []
    for c in range(8):
        b, g = c // 2, c % 2
        in_maps.append({
            "xT": np.ascontiguousarray(x[b].T).astype(bf),
            "wqk": np.ascontiguousarray(
                np.concatenate(
                    [w_qkv[:, 512 * g : 512 * (g + 1)],
                     w_qkv[:, 1024 + 512 * g : 1024 + 512 * (g + 1)]], axis=1
                )).astype(bf),
            "wv": np.ascontiguousarray(
                w_qkv[:, 2048 + 512 * g : 2048 + 512 * (g + 1)]).astype(bf),
            "wp": np.ascontiguousarray(
                w_proj[512 * g : 512 * (g + 1), :]).astype(bf),
        })
    return in_maps


def kernel(x, w_qkv, w_proj, b_proj):
    from concourse.bass_utils import run_bass_kernel_spmd

    x = np.asarray(x, dtype=np.float32)
    w_qkv = np.asarray(w_qkv, dtype=np.float32)
    w_proj = np.asarray(w_proj, dtype=np.float32)
    b_proj = np.asarray(b_proj, dtype=np.float32)

    nc = get_nc()
    in_maps = make_in_maps(x, w_qkv, w_proj)
    res = run_bass_kernel_spmd(nc, in_maps, list(range(8))).results

    out = np.zeros((B, N, DIM), dtype=np.float32)
    for c in range(8):
        out[c // 2] += res[c]["y"]
    return out + b_proj


# revision 17
# speedup vs baseline: 1.5100x; 1.0083x over previous
# Trainium2 Bass kernel for nn_Attention_80779744903426
#
# Reference computation (b=4, n=2048, c=1024, h=16, d=64):
#   qkv = x @ w_qkv ; split to q,k,v per head
#   attn = softmax(q k^T / sqrt(c)) ; out = (attn v) concat ; y = out @ w_proj + b_proj
#
# Sharding (8 cores): data-parallel over batch (4) x tensor-parallel over
# head-groups (2 groups of 8 heads, Megatron-style). Each core computes a
# partial y for its batch from its 8 heads; host sums the two partials per
# batch and adds b_proj.
#
# Per-core program (all matmuls bf16, fp32 PSUM accumulation):
#   A) qk^T = wqk^T @ x^T staged to DRAM bf16 (Q^T rows 0:512, K^T rows
#      512:1024); V = x @ wv -> SBUF bf16 with a ones column appended.
#      Pass 1 = V + K^T/Q^T of head-pair 0; the remaining projection chains
#      are interleaved into pair-0's attention iterations so the PE fills
#      the ACT(exp)-bound stretches.
#   B) per head pair and q-chunk of 512, software-pipelined:
#      S^T[k,q] = K^T_h(stationary, row-tiled K=64, A/B heads interleaved on
#      row groups) x Q^T_h(moving); exp via ACT over 3-bank PSUM batches
#      (softmax scale folded into the activation), bf16 out;
#      O'[65,q] = [V_h | 1]^T @ P~^T over 16 k-tiles (ones column = fused
#      softmax denominator). PV of the previous (pair,chunk) is interleaved
#      between S batches of the current one so the in-order PE queue never
#      stalls (stalls re-throttle the PE clock via HAM). Normalization:
#      copy O' out of PSUM, fast-reciprocal of the sums row (partition 0),
#      partition-broadcast via a DRAM-bounce DMA on the gpsimd queue,
#      multiply.
#   C) y = O^T(stationary) @ wp(moving) over 4 o-tiles; interleaved into the
#      last pair's iterations per q-chunk.

import numpy as np

DIM = 1024
N = 2048
B = 4
NH = 16
HD = 64
SCALE = 1.0 / DIM**0.5

HPC = 8            # heads per core
PAIRS = HPC // 2   # head pairs (row-tiled together)
CT = 8             # contraction tiles over c=1024
NT = 16            # n tiles of 128
ACH = 512          # phase-A n-chunk
QCH = 512          # phase-B q-chunk
NQC = N // QCH     # 4 q-chunks
KT = 16            # k tiles of 128 in attention

S_BATCHES = [(0, 3), (3, 3), (6, 3), (9, 3), (12, 2), (14, 2)]

_CACHE = {}


def _build_nc():
    import concourse.bass as bass
    from concourse import bacc, mybir, tile

    f32 = mybir.dt.float32
    bf16 = mybir.dt.bfloat16
    EXP = mybir.ActivationFunctionType.Exp

    nc = bacc.Bacc("TRN2", target_bir_lowering=False, debug=False)

    xT_d = nc.dram_tensor("xT", [DIM, N], bf16, kind="ExternalInput").ap()
    wqk_d = nc.dram_tensor("wqk", [DIM, 1024], bf16, kind="ExternalInput").ap()
    wv_d = nc.dram_tensor("wv", [DIM, 512], bf16, kind="ExternalInput").ap()
    wp_d = nc.dram_tensor("wp", [512, DIM], bf16, kind="ExternalInput").ap()
    y_d = nc.dram_tensor("y", [N, DIM], f32, kind="ExternalOutput").ap()

    with tile.TileContext(nc) as tc:
        with (
            tc.tile_pool(name="p16", bufs=5) as p16,      # 16KB slots: ptiles / xt
            tc.tile_pool(name="wqk", bufs=1) as wqkp,
            tc.tile_pool(name="wv", bufs=1) as wvp,
            tc.tile_pool(name="wp", bufs=1) as wpp,
            tc.tile_pool(name="v", bufs=1) as vp,
            tc.tile_pool(name="ot", bufs=1) as otp,
            tc.tile_pool(name="kt", bufs=2) as ktp,
            tc.tile_pool(name="qt", bufs=2) as qtp,
            tc.tile_pool(name="misc", bufs=2) as miscp,
            tc.tile_pool(name="ps", bufs=1, space="PSUM") as psp,
            tc.tile_pool(name="dram", bufs=1, space="DRAM") as dp,
        ):
            qkT_d = dp.tile([DIM, N], bf16, name="qkT_stage")
            # ---- static tiles ----
            wqk_sb = wqkp.tile([128, CT, 1024], bf16)
            for ct in range(CT):
                nc.sync.dma_start(wqk_sb[:, ct, :], wqk_d[128 * ct : 128 * (ct + 1), :])
            wv_sb = wvp.tile([128, CT, 512], bf16)
            for ct in range(CT):
                nc.sync.dma_start(wv_sb[:, ct, :], wv_d[128 * ct : 128 * (ct + 1), :])
            wp_sb = wpp.tile([128, 4, 1024], bf16)
            for ot in range(4):
                nc.sync.dma_start(wp_sb[:, ot, :], wp_d[128 * ot : 128 * (ot + 1), :])

            v_sb = vp.tile([128, NT, HPC, HD + 1], bf16)  # [k-part, k-tile, head, d | 1]
            nc.vector.memset(v_sb[:, :, :, HD], 1.0)

            ot_sb = otp.tile([128, PAIRS, N], bf16)  # O^T rows: pair p = rows 128p..

            xT_r = xT_d.rearrange("(t p) n -> p t n", p=128)

            # ---- phase A helpers ----
            def emit_qkt_chain(xt, mt, ach):
                qps = psp.tile([128, 512], f32, tag="acc", bufs=2, name="qps")
                for ct in range(CT):
                    nc.tensor.matmul(qps, wqk_sb[:, ct, 128 * mt : 128 * (mt + 1)],
                                     xt[:, ct, :], start=(ct == 0), stop=(ct == CT - 1))
                stg = miscp.tile([128, 512], bf16, tag="stg", bufs=3, name="stg")
                nc.vector.tensor_copy(stg, qps)
                nc.sync.dma_start(
                    qkT_d[128 * mt : 128 * (mt + 1), ACH * ach : ACH * (ach + 1)], stg
                )

            def load_xt(ach):
                xt = p16.tile([128, CT, ACH], bf16, tag="big16", name="xt")
                nc.sync.dma_start(xt, xT_r[:, :, ACH * ach : ACH * (ach + 1)])
                return xt

            def emit_v_group(ach):
                xt = load_xt(ach)
                for sub in range(ACH // 128):
                    nt = (ACH // 128) * ach + sub
                    vps = psp.tile([128, 512], f32, tag="acc", bufs=2, name="vps")
                    for ct in range(CT):
                        nc.tensor.matmul(vps, xt[:, ct, 128 * sub : 128 * (sub + 1)],
                                         wv_sb[:, ct, :], start=(ct == 0),
                                         stop=(ct == CT - 1))
                    nc.vector.tensor_copy(
                        v_sb[:, nt, :, 0:HD],
                        vps.rearrange("p (h d) -> p h d", h=HPC),
                    )

            # ---- phase A pass 1: K^T/Q^T for pair 0 only (fast path to B) ----
            for ach in range(N // ACH):
                xt = load_xt(ach)
                emit_qkt_chain(xt, 4, ach)
                emit_qkt_chain(xt, 0, ach)

            # ---- phase B (+ interleaved A pass 2 and phase C) ----
            PV_SEGS = [(0, 3), (3, 3), (6, 3), (9, 3), (12, 2), (14, 2)]
            PASS2_MTS = [5, 1, 6, 2, 7, 3]

            def emit_pv_segment(st, seg):
                p0, ptl, opsl = st
                k0, kn = PV_SEGS[seg]
                for hh in range(2):
                    h = 2 * p0 + hh
                    for k in range(k0, k0 + kn):
                        nc.tensor.matmul(opsl[hh], v_sb[:, k, h, :],
                                         ptl[hh][:, k, :],
                                         start=(k == 0), stop=(k == KT - 1))

            def emit_norm(st, qc0):
                # Copy O' out of PSUM first so the PSUM slot recycles without
                # waiting for the reciprocal/broadcast chain. Bounce DMAs ride
                # the gpsimd SWDGE queue so they never head-of-line-block the
                # sync queue carrying bulk loads.
                p0, ptl, opsl = st
                for hh in range(2):
                    ops = opsl[hh]
                    ostg = miscp.tile([HD, QCH], f32, tag="ostg", bufs=4,
                                      name="ostg")
                    nc.vector.tensor_copy(ostg, ops[0:HD, :])
                    # denominator row staged to partition 0: the custom-DVE
                    # reciprocal_approx_fast misreads non-zero base partitions
                    den = miscp.tile([1, QCH], f32, tag="den", bufs=4, name="den")
                    nc.vector.tensor_copy(den, ops[HD : HD + 1, :])
                    rcp = miscp.tile([1, QCH], f32, tag="rcp", bufs=4, name="rcp")
                    nc.vector.reciprocal_approx_fast(rcp, den)
                    rcp_d = dp.tile([1, QCH], f32, tag="rcpd", bufs=4, name="rcpd")
                    nc.gpsimd.dma_start(rcp_d, rcp)
                    bc = miscp.tile([64, QCH], f32, tag="bc", bufs=4, name="bc")
                    rap = rcp_d[:]
                    nc.gpsimd.dma_start(
                        bc,
                        bass.AP(tensor=rap.tensor, offset=rap.offset,
                                ap=[[0, 64]] + list(rap.ap[1:])),
                    )
                    nc.vector.tensor_mul(
                        ot_sb[64 * hh : 64 * (hh + 1), p0, QCH * qc0 : QCH * (qc0 + 1)],
                        ostg,
                        bc,
                    )

            def emit_proj_chunk(qc0):
                # y columns for q-chunk qc0 (needs ot_sb[:, :, chunk] complete)
                for sub in range(QCH // 128):
                    nt = (QCH // 128) * qc0 + sub
                    for yc in range(2):
                        yps = psp.tile([128, 512], f32, tag="acc", bufs=2, name="yps")
                        for ot in range(4):
                            nc.tensor.matmul(
                                yps, ot_sb[:, ot, 128 * nt : 128 * (nt + 1)],
                                wp_sb[:, ot, 512 * yc : 512 * (yc + 1)],
                                start=(ot == 0), stop=(ot == 3))
                        stg = miscp.tile([128, 512], f32, tag="ystg", bufs=2,
                                         name="ystg")
                        nc.vector.tensor_copy(stg, yps)
                        nc.sync.dma_start(
                            y_d[128 * nt : 128 * (nt + 1), 512 * yc : 512 * (yc + 1)],
                            stg,
                        )

            def make_pass2_thunks(ach):
                # split one pass-2 chunk (xt load + 6 chains) into 3 thunks
                box = {}

                def t0():
                    box["xt"] = load_xt(ach)
                    for mt in PASS2_MTS[0:2]:
                        emit_qkt_chain(box["xt"], mt, ach)

                def t1():
                    for mt in PASS2_MTS[2:4]:
                        emit_qkt_chain(box["xt"], mt, ach)

                def t2():
                    for mt in PASS2_MTS[4:6]:
                        emit_qkt_chain(box["xt"], mt, ach)

                return [t0, t1, t2]

            # Extra PE work injected into pair-0's ACT(exp)-bound iterations:
            # (p0,qc0) computes V; (p0,qc1..3) stage pass-2 chunks 0..2; the
            # final pass-2 chunk is emitted standalone before pair 1 (it
            # gates pair-1's kt DMA, so it must precede pair-1's S batches in
            # the in-order PE queue).
            extras_by_iter = {
                (0, 0): [lambda a=a: emit_v_group(a) for a in range(4)],
                (0, 1): make_pass2_thunks(0),
                (0, 2): make_pass2_thunks(1),
                (0, 3): make_pass2_thunks(2),
            }

            pv_st = None
            pv_qc = None
            for p in range(PAIRS):
                kt_sb = ktp.tile([128, N], bf16, name="kt_sb")
                nc.sync.dma_start(kt_sb, qkT_d[512 + 128 * p : 512 + 128 * (p + 1), :])
                for qc in range(NQC):
                    qt_sb = qtp.tile([128, QCH], bf16, name="qt_sb")
                    nc.sync.dma_start(
                        qt_sb, qkT_d[128 * p : 128 * (p + 1), QCH * qc : QCH * (qc + 1)]
                    )
                    extras = extras_by_iter.get((p, qc), [])
                    ptiles = [
                        p16.tile([128, KT, QCH], bf16, tag="big16", name=f"pt{hh}")
                        for hh in range(2)
                    ]
                    for bi, (b0, bn) in enumerate(S_BATCHES):
                        sps = [
                            psp.tile([128, 3, QCH], f32, tag="sb3", bufs=2,
                                     name=f"sps{hh}")
                            for hh in range(2)
                        ]
                        for i in range(bn):
                            k = b0 + i
                            for hh in range(2):
                                sl = slice(64 * hh, 64 * (hh + 1))
                                nc.tensor.matmul(
                                    sps[hh][:, i, :],
                                    kt_sb[sl, 128 * k : 128 * (k + 1)],
                                    qt_sb[sl, :], start=True, stop=True)
                        for hh in range(2):
                            nc.scalar.activation(
                                out=ptiles[hh][:, b0 : b0 + bn, :],
                                in_=sps[hh][:, 0:bn, :],
                                func=EXP,
                                scale=float(SCALE),
                            )
                        if pv_st is not None:
                            emit_pv_segment(pv_st, bi)
                        if bi < len(extras):
                            extras[bi]()
                    if pv_st is not None:
                        emit_norm(pv_st, pv_qc)
                        if pv_st[0] == PAIRS - 1:
                            emit_proj_chunk(pv_qc)
                    opsl = [
                        psp.tile([HD + 1, QCH], f32, tag="acc", bufs=2,
                                 name=f"ops{hh}")
                        for hh in range(2)
                    ]
                    pv_st = (p, ptiles, opsl)
                    pv_qc = qc
                    if p == 0 and qc == NQC - 1:
                        # final pass-2 chunk must complete before pair 1's
                        # kt DMA (emitted next) can be satisfied
                        xt = load_xt(3)
                        for mt in PASS2_MTS:
                            emit_qkt_chain(xt, mt, 3)
            # drain the last (pair, q-chunk)
            for seg in range(len(PV_SEGS)):
                emit_pv_segment(pv_st, seg)
            emit_norm(pv_st, pv_qc)
            emit_proj_chunk(pv_qc)

    nc.compile()
    return nc


def get_nc():
    if "nc" not in _CACHE:
        _CACHE["nc"] = _build_nc()
    return _CACHE["nc"]


def make_in_maps(x, w_qkv, w_proj):
    import ml_dtypes

    bf = ml_dtypes.bfloat16
    in_maps = []
    for c in range(8):
        b, g = c // 2, c % 2
        in_maps.append({
            "xT": np.ascontiguousarray(x[b].T).astype(bf),
            "wqk": np.ascontiguousarray(
                np.concatenate(
                    [w_qkv[:, 512 * g : 512 * (g + 1)],
                     w_qkv[:, 1024 + 512 * g : 1024 + 512 * (g + 1)]], axis=1
                )).astype(bf),
            "wv": np.ascontiguousarray(
                w_qkv[:, 2048 + 512 * g : 2048 + 512 * (g + 1)]).astype(bf),
            "wp": np.ascontiguousarray(
                w_proj[512 * g : 512 * (g + 1), :]).astype(bf),
        })
    return in_maps


def kernel(x, w_qkv, w_proj, b_proj):
    from concourse.bass_utils import run_bass_kernel_spmd

    x = np.asarray(x, dtype=np.float32)
    w_qkv = np.asarray(w_qkv, dtype=np.float32)
    w_proj = np.asarray(w_proj, dtype=np.float32)
    b_proj = np.asarray(b_proj, dtype=np.float32)

    nc = get_nc()
    in_maps = make_in_maps(x, w_qkv, w_proj)
    res = run_bass_kernel_spmd(nc, in_maps, list(range(8))).results

    out = np.zeros((B, N, DIM), dtype=np.float32)
    for c in range(8):
        out[c // 2] += res[c]["y"]
    return out + b_proj
